# revision 1
# baseline (speedup 1.0000x reference)
"""AdaProj loss kernel for 8 TRN2 NeuronCores (Bass/Tile).

Math (per reference):
  xn = l2norm(x, 1); Wn = l2norm(W, 2); coef[b,c,s] = xn . Wn[c,s]
  q1 = sum_s coef^2 ; q2 = coef^T G_c coef (G = Wn Wn^T)
  logits = q1/sqrt(q2); loss = mean_b( lse_c(s*logits) - s*logits[b,lab] )

All normalizations are algebraically folded away; with RAW inputs
(craw = W^T-rows . x, rsq = |W row|^2, rinv = 1/rsq, v = rinv*craw):
  q1r = sum_s v*craw ; q2r = sum_s' v*(Graw v) ; logits = q1r/sqrt(q2r*|x|^2)
so the device needs only one reciprocal per row block and one
Ln/Exp pair (plus softmax exp) -- no sqrt, no normalize passes.

Sharding: class-parallel, C=512 -> 64 classes/core; W and x shards are sent
host-transposed (wT [E,CS], xT [E,B]). Each core returns
  out[0,:] = sum_{c in shard} exp(s*logits - s) ; out[1,:] = sum_c y*logits
Host: loss = mean( log(sum_i se_i) + s - s*sum_i t0_i ).
"""

import sys

for _p in ("/opt/trn_rl_repo",):
    if _p not in sys.path:
        sys.path.insert(0, _p)

import ml_dtypes
import numpy as np

import bass_rust
import concourse.bass as bass
import concourse.tile as tile
from concourse import mybir
from concourse.bass_utils import run_bass_kernel_spmd

FP32 = mybir.dt.float32
BF16 = mybir.dt.bfloat16

B, C, S, E = 1024, 512, 32, 128
NCORES = 8
C_LOC = C // NCORES            # 64 classes per core
CS = C_LOC * S                 # 2048 rows of the local basis
NT = CS // 128                 # 16 cs-tiles of 128 rows (4 classes each)
NP = NT // 2                   # 8 pairs of cs-tiles
NB = B // 512
CPT = 128 // S                 # classes per cs-tile = 4


def build_nc(s_val: float, n_iters: int = 1) -> bass.Bass:
    nc = bass.Bass()

    w_ext = nc.declare_dram_parameter("wT", [E, CS], FP32, isOutput=False)
    x_ext = nc.declare_dram_parameter("xT", [E, B], FP32, isOutput=False)
    yt_ext = nc.declare_dram_parameter("yt", [C_LOC, B], BF16, isOutput=False)
    mask_ext = nc.declare_dram_parameter("mask", [128, 128], FP32, isOutput=False)
    id_ext = nc.declare_dram_parameter("ident", [128, 128], FP32, isOutput=False)
    ind_ext = nc.declare_dram_parameter("indp", [128, 124], BF16, isOutput=False)
    ones_ext = nc.declare_dram_parameter("ones", [128, 1], BF16, isOutput=False)
    out_ext = nc.declare_dram_parameter("out", [2, B], FP32, isOutput=True)

    Mult = mybir.AluOpType.mult
    Exp = mybir.ActivationFunctionType.Exp
    Ln = mybir.ActivationFunctionType.Ln
    Copy = mybir.ActivationFunctionType.Copy

    with tile.TileContext(nc) as tc:
        with (
            tc.tile_pool(name="persist", bufs=1) as pp,
            tc.tile_pool(name="xload", bufs=2) as p_x,
            tc.tile_pool(name="wload", bufs=2) as p_w,
            tc.tile_pool(name="wsmall", bufs=3) as p_ws,
            tc.tile_pool(name="big", bufs=2) as p_big,
            tc.tile_pool(name="fin", bufs=2) as p_f,
            tc.tile_pool(name="psCH", bufs=1, space="PSUM") as ps_ch,
            tc.tile_pool(name="psQ", bufs=1, space="PSUM") as ps_q,
            tc.tile_pool(name="psF", bufs=1, space="PSUM") as ps_f,
        ):
            # ---- constants, loaded once
            mask = pp.tile([128, 128], FP32, tag="mask")
            nc.sync.dma_start(out=mask[:], in_=mask_ext[:])
            ident = pp.tile([128, 128], FP32, tag="ident")
            nc.sync.dma_start(out=ident[:], in_=id_ext[:])
            indp = pp.tile([128, 124], BF16, tag="indp")
            nc.sync.dma_start(out=indp[:], in_=ind_ext[:])
            ones = pp.tile([128, 1], BF16, tag="ones")
            nc.sync.dma_start(out=ones[:], in_=ones_ext[:])
            yt = pp.tile([C_LOC, B], BF16, tag="yt")
            nc.sync.dma_start(out=yt[:], in_=yt_ext[:])
            negs = pp.tile([128, 1], FP32, tag="negs")
            nc.vector.memset(negs[:], -s_val)

            for it in range(n_iters):
                # ---- loads: W one DMA + one cast; x one DMA + one cast
                wTf = p_w.tile([128, CS], FP32, tag="wTf")
                nc.sync.dma_start(out=wTf[:], in_=w_ext[:])
                wTb = p_w.tile([128, CS], BF16, tag="wTb")
                nc.gpsimd.tensor_copy(wTb[:], wTf[:])

                xTf = p_x.tile([128, B], FP32, tag="xTf")
                nc.sync.dma_start(out=xTf[:], in_=x_ext[:])
                xTb = p_x.tile([128, B], BF16, tag="xTb")
                nc.gpsimd.tensor_copy(xTb[:], xTf[:])

                # |x|^2 per column: square (bf16 2x), ones-matmul, bcast
                sqx = p_x.tile([128, B], BF16, tag="sqx")
                nc.vector.scalar_tensor_tensor(
                    out=sqx[:], in0=xTb[:], scalar=1.0, in1=xTb[:],
                    op0=Mult, op1=Mult,
                )
                xssr = p_x.tile([1, B], FP32, tag="xssr")
                pxs = ps_f.tile([1, B], FP32, tag="psF")
                for nb in range(NB):
                    nc.tensor.matmul(
                        pxs[:, nb * 512 : (nb + 1) * 512], lhsT=ones[:],
                        rhs=sqx[:, nb * 512 : (nb + 1) * 512],
                        start=True, stop=True,
                    )
                nc.scalar.copy(out=xssr[:], in_=pxs[:])
                xssB = p_x.tile([128, B], FP32, tag="xssB")
                _src = xssr[0:1, :]
                nc.sync.dma_start(
                    out=xssB[:],
                    in_=bass.AP(
                        tensor=_src.tensor, offset=_src.offset,
                        ap=[[1, 1], [0, 128], [1, B]],
                    ),
                )

                # ---- per pair of cs-tiles
                qps = ps_q.tile([128, B], FP32, tag="q")
                for p in range(NP):
                    ta, tb = 2 * p, 2 * p + 1
                    wsl_a = wTb[:, ta * 128 : (ta + 1) * 128]
                    wsl_b = wTb[:, tb * 128 : (tb + 1) * 128]
                    pc = ps_ch.tile([128, 2 * B], FP32, tag="psCH")

                    # raw Gram blocks -> rsq/rinv + masked gblk
                    nc.tensor.matmul(
                        pc[:, 0:128], lhsT=wsl_a, rhs=wsl_a, start=True, stop=True
                    )
                    nc.tensor.matmul(
                        pc[:, 128:256], lhsT=wsl_b, rhs=wsl_b, start=True, stop=True
                    )
                    rsqp = p_ws.tile([128, 2], FP32, tag="rsqp")
                    dg = p_ws.tile([128, 128], BF16, tag="dg")
                    nc.vector.scalar_tensor_tensor(
                        out=dg[:], in0=pc[:, 0:128], scalar=1.0, in1=ident[:],
                        op0=Mult, op1=Mult, accum_out=rsqp[:, 0:1],
                    )
                    dg2 = p_ws.tile([128, 128], BF16, tag="dg2")
                    nc.vector.scalar_tensor_tensor(
                        out=dg2[:], in0=pc[:, 128:256], scalar=1.0, in1=ident[:],
                        op0=Mult, op1=Mult, accum_out=rsqp[:, 1:2],
                    )
                    rinvp = p_ws.tile([128, 2], FP32, tag="rinvp")
                    nc.vector.reciprocal(out=rinvp[:], in_=rsqp[:])
                    gba_t = p_ws.tile([128, 128], BF16, tag="gba")
                    nc.vector.tensor_tensor(
                        out=gba_t[:], in0=pc[:, 0:128], in1=mask[:], op=Mult
                    )
                    gbb_t = p_ws.tile([128, 128], BF16, tag="gbb")
                    nc.vector.tensor_tensor(
                        out=gbb_t[:], in0=pc[:, 128:256], in1=mask[:], op=Mult
                    )
                    gba = gba_t[:]
                    gbb = gbb_t[:]

                    # craw (overwrite psCH) ; v = rinv*craw ; e = v*craw
                    for h, wsl in ((0, wsl_a), (1, wsl_b)):
                        for nb in range(NB):
                            nc.tensor.matmul(
                                pc[:, h * B + nb * 512 : h * B + (nb + 1) * 512],
                                lhsT=wsl,
                                rhs=xTb[:, nb * 512 : (nb + 1) * 512],
                                start=True, stop=True,
                            )
                    vp = p_big.tile([128, 2 * B], BF16, tag="vp")
                    nc.scalar.activation(
                        out=vp[:, 0:B], in_=pc[:, 0:B], func=Copy,
                        scale=rinvp[:, 0:1],
                    )
                    nc.scalar.activation(
                        out=vp[:, B : 2 * B], in_=pc[:, B : 2 * B], func=Copy,
                        scale=rinvp[:, 1:2],
                    )
                    # e = rsq*v^2 (= v*craw) at 2x from SBUF
                    ep = p_big.tile([128, 2 * B], BF16, tag="ep")
                    nc.vector.scalar_tensor_tensor(
                        out=ep[:, 0:B], in0=vp[:, 0:B], scalar=rsqp[:, 0:1],
                        in1=vp[:, 0:B], op0=Mult, op1=Mult,
                    )
                    nc.vector.scalar_tensor_tensor(
                        out=ep[:, B : 2 * B], in0=vp[:, B : 2 * B],
                        scalar=rsqp[:, 1:2],
                        in1=vp[:, B : 2 * B], op0=Mult, op1=Mult,
                    )

                    # h = Graw v (overwrite psCH) ; ch = v*h
                    for h, gb in ((0, gba), (1, gbb)):
                        for nb in range(NB):
                            nc.tensor.matmul(
                                pc[:, h * B + nb * 512 : h * B + (nb + 1) * 512],
                                lhsT=gb,
                                rhs=vp[:, h * B + nb * 512 : h * B + (nb + 1) * 512],
                                start=True, stop=True,
                            )
                    chp = p_big.tile([128, 2 * B], BF16, tag="chp")
                    nc.vector.tensor_tensor(out=chp[:], in0=vp[:], in1=pc[:], op=Mult)

                    # segmented reduces (PSUM-accumulated across tiles)
                    for h, t in ((0, ta), (1, tb)):
                        ind_t = indp[:, 60 - CPT * t : 124 - CPT * t]
                        for nb in range(NB):
                            nc.tensor.matmul(
                                qps[0:64, nb * 512 : (nb + 1) * 512],
                                lhsT=ind_t,
                                rhs=ep[:, h * B + nb * 512 : h * B + (nb + 1) * 512],
                                start=(t == 0), stop=(t == NT - 1),
                            )
                            nc.tensor.matmul(
                                qps[64:128, nb * 512 : (nb + 1) * 512],
                                lhsT=ind_t,
                                rhs=chp[:, h * B + nb * 512 : h * B + (nb + 1) * 512],
                                start=(t == 0), stop=(t == NT - 1),
                            )

                # ---- logits, softmax partials, output
                qT = p_f.tile([128, B], FP32, tag="qT")
                nc.scalar.copy(out=qT[:], in_=qps[:])
                q2s = p_f.tile([C_LOC, B], FP32, tag="q2s")
                nc.vector.tensor_tensor(
                    out=q2s[:], in0=qT[64:128, :], in1=xssB[64:128, :], op=Mult
                )
                invs = p_f.tile([C_LOC, B], FP32, tag="invs")
                nc.scalar.activation(out=invs[:], in_=q2s[:], func=Ln)
                nc.scalar.activation(out=invs[:], in_=invs[:], func=Exp, scale=-0.5)
                logitsT = p_f.tile([C_LOC, B], BF16, tag="logitsT")
                nc.vector.tensor_tensor(
                    out=logitsT[:], in0=qT[0:64, :], in1=invs[:], op=Mult
                )
                expz = p_f.tile([C_LOC, B], BF16, tag="expz")
                nc.scalar.activation(
                    out=expz[:], in_=logitsT[:], func=Exp, scale=s_val,
                    bias=negs[0:C_LOC, :],
                )
                tl = p_f.tile([C_LOC, B], BF16, tag="tl")
                nc.gpsimd.tensor_tensor(
                    out=tl[:], in0=yt[:], in1=logitsT[:], op=Mult
                )

                outse = p_f.tile([1, B], FP32, tag="outse")
                outt0 = p_f.tile([1, B], FP32, tag="outt0")
                pse = ps_f.tile([1, B], FP32, tag="psF")
                for nb in range(NB):
                    nc.tensor.matmul(
                        pse[:, nb * 512 : (nb + 1) * 512], lhsT=ones[0:C_LOC, :],
                        rhs=expz[:, nb * 512 : (nb + 1) * 512],
                        start=True, stop=True,
                    )
                nc.scalar.copy(out=outse[:], in_=pse[:])
                pt0 = ps_f.tile([1, B], FP32, tag="psF")
                for nb in range(NB):
                    nc.tensor.matmul(
                        pt0[:, nb * 512 : (nb + 1) * 512], lhsT=ones[0:C_LOC, :],
                        rhs=tl[:, nb * 512 : (nb + 1) * 512],
                        start=True, stop=True,
                    )
                nc.vector.tensor_copy(outt0[:], pt0[:])
                nc.sync.dma_start(out=out_ext[0:1, :], in_=outse[:])
                nc.sync.dma_start(out=out_ext[1:2, :], in_=outt0[:])

    # Split multi-wait sync_info into EventSemaphore instructions (HW allows
    # only 1 wait per instruction in this toolchain's walrus).
    bass_rust.move_matmul_waits_to_ldweights(nc.m)
    bass_rust.generate_event_semaphores(nc)
    return nc


def make_aux():
    mask = np.zeros((128, 128), dtype=np.float32)
    for j in range(CPT):
        mask[j * S : (j + 1) * S, j * S : (j + 1) * S] = 1.0
    ident = np.eye(128, dtype=np.float32)
    indp = np.zeros((128, 124), dtype=ml_dtypes.bfloat16)
    for k in range(128):
        indp[k, 60 + k // S] = 1.0
    ones = np.ones((128, 1), dtype=ml_dtypes.bfloat16)
    return mask, ident, indp, ones


def make_in_maps(x, y, W):
    mask, ident, indp, ones = make_aux()
    xT = np.ascontiguousarray(x.T)
    in_maps = []
    for i in range(NCORES):
        wT_i = np.ascontiguousarray(
            W[i * C_LOC : (i + 1) * C_LOC].reshape(CS, E).T
        )
        yt_i = np.ascontiguousarray(
            y[:, i * C_LOC : (i + 1) * C_LOC].T
        ).astype(ml_dtypes.bfloat16)
        in_maps.append(
            {
                "wT": wT_i, "xT": xT, "yt": yt_i,
                "mask": mask, "ident": ident, "indp": indp, "ones": ones,
            }
        )
    return in_maps


def combine(outs, s_val):
    se = np.zeros(B, dtype=np.float64)
    t0 = np.zeros(B, dtype=np.float64)
    for o in outs:
        se += o[0]
        t0 += o[1]
    return np.float32(np.mean(np.log(se) + s_val - s_val * t0))


_CACHE = {}


def kernel(x, y, W, s, **_unused):
    x = np.ascontiguousarray(np.asarray(x, dtype=np.float32))
    y = np.asarray(y, dtype=np.float32)
    W = np.asarray(W, dtype=np.float32)
    s_val = float(np.asarray(s))

    key = ("v4", s_val)
    nc = _CACHE.get(key)
    if nc is None:
        nc = build_nc(s_val)
        _CACHE[key] = nc

    in_maps = make_in_maps(x, y, W)
    res = run_bass_kernel_spmd(nc, in_maps, core_ids=list(range(NCORES)))
    outs = [np.asarray(r["out"], dtype=np.float64) for r in res.results]
    return combine(outs, s_val)


if __name__ == "__main__":
    rng = np.random.default_rng(0)
    x = rng.standard_normal((B, E), dtype=np.float32)
    lab = rng.integers(0, C, size=B)
    y = np.eye(C, dtype=np.float32)[lab]
    W = rng.uniform(-0.1, 0.1, size=(C, S, E)).astype(np.float32)
    s = np.float32(np.sqrt(2.0) * np.log(C - 1.0))
    print(kernel(x=x, y=y, W=W, s=s))

